# revision 10
# baseline (speedup 1.0000x reference)
"""Bahdanau-style attention kernel for Trainium2, SPMD over 8 NeuronCores.

Shapes (full): encoder_output [256, 196, 2048], decoder_hidden [256, 512],
W_enc [2048, 512], b_enc [512], W_dec [512, 512], b_dec [512],
W_att [512, 1], b_att [1].
Outputs: (context_vector [256, 2048], alpha [256, 196]).

Sharding: data-parallel over batch, 32 batches per core; weights replicated.

Per-core algorithm (B=32 local batches, R = 32*196 = 6272 flattened rows):
  - enc rows are processed in 13 row-chunks (12x512 + 1x128), tiled [128, 2048].
  - PE transposes build encT [e, p] chunks; fp32r matmuls with W_enc stationary
    accumulate enc_attT [a_tile, p_chunk] in PSUM over 16 e-tiles.
  - ScalarE applies relu(x + dec_attT[:, b]) with the per-batch decoder bias
    (b_enc + b_dec folded in) as a per-partition scalar.
  - fp32r matmuls with W_att stationary give scoresT [1, p_chunk].
  - softmax uses the shift-invariance of softmax to skip the max subtraction
    (scores are O(1) here): context = (sum_p exp(s) * enc) / sum_p exp(s), so
    the context accumulates UNNORMALIZED per chunk with no global barrier.
  - exp(scores) is repartitioned to row-major [128, tile] via tiny PE
    transposes; per enc-tile sparse coefficient matrices A_k [128, 32]
    (column b = exp scores of batch b's rows in tile k) let one PSUM bank
    accumulate all 4 e-chunks of the numerator (partition-packed, 32 rows per
    e-chunk via tile_position).
  - scores also bounce through DRAM (row-major) and come back batch-major
    [32, 196] for the alpha output; exp+row-sum there gives the normalizer.
"""

import os
import numpy as np

B, P, E, A, D = 256, 196, 2048, 512, 512
NCORES = 8
BC = B // NCORES          # 32 batches per core
R = BC * P                # 6272 rows per core
NT = R // 128             # 49 row-tiles of 128
NE = E // 128             # 16 e-tiles
NA = A // 128             # 4 a-tiles
CHUNKS = [(c * 512, 512) for c in range(R // 512)] + (
    [(R - R % 512, R % 512)] if R % 512 else []
)

_CACHE = {}


def _segments(r0, nr):
    """Yield (batch, off0, off1) covering rows [r0, r0+nr) split at batch bounds."""
    r = r0
    out = []
    while r < r0 + nr:
        b = r // P
        e = min((b + 1) * P, r0 + nr)
        out.append((b, r - r0, e - r0))
        r = e
    return out


def _build_nc():
    import concourse.bass as bass
    import concourse.tile as tile
    from concourse import bacc, mybir
    from concourse.masks import make_identity
    from contextlib import ExitStack

    f32 = mybir.dt.float32
    f32r = mybir.dt.float32r
    RELU = mybir.ActivationFunctionType.Relu
    EXP = mybir.ActivationFunctionType.Exp
    MULT = mybir.AluOpType.add  # placeholder, reassigned below
    MULT = mybir.AluOpType.mult
    ADD = mybir.AluOpType.add

    nc = bacc.Bacc("TRN2", target_bir_lowering=False, debug=False, num_devices=NCORES)

    enc_d = nc.dram_tensor("encoder_output", [BC, P, E], f32, kind="ExternalInput")
    dec_d = nc.dram_tensor("decoder_hidden", [BC, D], f32, kind="ExternalInput")
    wenc_d = nc.dram_tensor("W_enc", [E, A], f32, kind="ExternalInput")
    benc_d = nc.dram_tensor("b_enc", [A], f32, kind="ExternalInput")
    wdec_d = nc.dram_tensor("W_dec", [D, A], f32, kind="ExternalInput")
    bdec_d = nc.dram_tensor("b_dec", [D], f32, kind="ExternalInput")
    watt_d = nc.dram_tensor("W_att", [A, 1], f32, kind="ExternalInput")
    ctx_d = nc.dram_tensor("context_vector", [BC, E], f32, kind="ExternalOutput")
    alpha_d = nc.dram_tensor("alpha", [BC, P], f32, kind="ExternalOutput")

    enc_flat = enc_d.ap().flatten_outer_dims()  # [R, E]

    # maskbank[k, p, b] = 1.0 iff global row 128k+p belongs to local batch b
    mask_np = np.zeros((NT, 128, BC), dtype=np.float32)
    for k in range(NT):
        for p_ in range(128):
            mask_np[k, p_, (128 * k + p_) // P] = 1.0
    mb_d = nc.inline_tensor(mask_np, name="maskbank")

    with tile.TileContext(nc) as tc, ExitStack() as ctx:
        consts = ctx.enter_context(tc.tile_pool(name="consts", bufs=1))
        encp = ctx.enter_context(tc.tile_pool(name="encp", bufs=10))
        encTp = ctx.enter_context(tc.tile_pool(name="encTp", bufs=3))
        combp = ctx.enter_context(tc.tile_pool(name="combp", bufs=2))
        smallp = ctx.enter_context(tc.tile_pool(name="smallp", bufs=2))
        apool = ctx.enter_context(tc.tile_pool(name="apool", bufs=3))
        ps_tr = ctx.enter_context(tc.tile_pool(name="ps_tr", bufs=2, space="PSUM"))
        ps_acc = ctx.enter_context(tc.tile_pool(name="ps_acc", bufs=1, space="PSUM"))
        ps_sc = ctx.enter_context(tc.tile_pool(name="ps_sc", bufs=1, space="PSUM"))
        dramp = ctx.enter_context(tc.tile_pool(name="dramp", bufs=1, space="DRAM"))

        identity = consts.tile([128, 128], f32)
        make_identity(nc, identity[:])
        identity_r = consts.tile([128, 128], f32r)
        nc.vector.tensor_copy(identity_r[:], identity[:])

        # ---- weight / bias loads -------------------------------------------
        wenc_r = consts.tile([128, NE, A], f32r)  # [p, etile, a], rounded by cast-DMA
        nc.gpsimd.dma_start(
            out=wenc_r[:], in_=wenc_d.ap().rearrange("(i p) a -> p i a", p=128)
        )
        wdec_sb = consts.tile([128, NA, A], f32)  # [p, dtile, a]
        nc.sync.dma_start(
            out=wdec_sb[:], in_=wdec_d.ap().rearrange("(i p) a -> p i a", p=128)
        )
        watt_r = consts.tile([128, NA], f32r)  # [p, atile]
        nc.gpsimd.dma_start(
            out=watt_r[:], in_=watt_d.ap().rearrange("(i p) o -> p (i o)", p=128)
        )
        benc_sb = consts.tile([1, A], f32)
        nc.sync.dma_start(out=benc_sb[:], in_=benc_d.ap().unsqueeze(0))
        bdec_sb = consts.tile([1, A], f32)
        nc.sync.dma_start(out=bdec_sb[:], in_=bdec_d.ap().unsqueeze(0))
        dec_sb = consts.tile([BC, D], f32)
        nc.sync.dma_start(out=dec_sb[:], in_=dec_d.ap())
        if not os.environ.get("SKIP_MASK"):
            mb_sb = consts.tile([128, NT, BC], f32)
            nc.sync.dma_start(out=mb_sb[:], in_=mb_d.ap().rearrange("k p b -> p k b"))

        # ---- prologue: dec_attT[a, b] = (dec @ W_dec + b_dec + b_enc).T ----
        bias_sb = consts.tile([1, A], f32)
        nc.vector.tensor_add(bias_sb[:], benc_sb[:], bdec_sb[:])
        biasT_ps = ps_sc.tile([128, NA], f32, tag="sc")
        for i in range(NA):
            nc.tensor.transpose(
                biasT_ps[:, i : i + 1], bias_sb[:, bass.ts(i, 128)], identity[:1, :1]
            )
        biasT_sb = consts.tile([128, NA], f32)
        nc.vector.tensor_copy(biasT_sb[:], biasT_ps[:])

        decT_sb = consts.tile([128, NA, BC], f32)  # [d_in_tile, dtile, b]
        for i in range(NA):
            tp = ps_tr.tile([128, 512], f32, tag="tr")
            nc.tensor.transpose(
                tp[:, :BC], dec_sb[:, bass.ts(i, 128)], identity[:BC, :BC]
            )
            nc.vector.tensor_copy(decT_sb[:, i, :], tp[:, :BC])

        decatt_sb = consts.tile([128, NA, BC], f32)  # [a_in_tile, atile, b]
        for t in range(NA):
            acc = ps_acc.tile([128, 512], f32, tag="acc0")
            for i in range(NA):
                nc.tensor.matmul(
                    acc[:, :BC],
                    wdec_sb[:, i, bass.ts(t, 128)],
                    decT_sb[:, i, :],
                    start=(i == 0),
                    stop=(i == NA - 1),
                )
            nc.vector.tensor_scalar(
                decatt_sb[:, t, :], acc[:, :BC], biasT_sb[:, t : t + 1], None, op0=ADD
            )

        # ---- main loop ------------------------------------------------------
        scores_dram = dramp.tile([1, R], f32)
        num_sb = consts.tile([BC, E], f32)  # unnormalized context accumulator
        nc.vector.memset(num_sb[:], 0.0)

        _nch = int(os.environ.get("KERNEL_NCHUNKS", "0")) or len(CHUNKS)
        for c, (r0, nr) in enumerate(CHUNKS[:_nch]):
            ntile = nr // 128
            etiles = []
            for t in range(ntile):
                et = encp.tile([128, E], f32r, tag="enc")
                nc.gpsimd.dma_start(
                    out=et[:], in_=enc_flat[r0 + 128 * t : r0 + 128 * (t + 1), :]
                )
                etiles.append(et)

            # step 1: enc_attT [a_tile, p] accumulated over e-tiles
            accs = [
                ps_acc.tile([128, 512], f32, tag=f"acc{a}", name=f"acc{a}")
                for a in range(NA)
            ]
            for j in range(NE):
                trp = ps_tr.tile([128, 512], f32r, tag="tr")
                for t in range(ntile):
                    nc.tensor.transpose(
                        trp[:, bass.ts(t, 128)],
                        etiles[t][:, bass.ts(j, 128)],
                        identity_r[:],
                    )
                encTj = encTp.tile([128, 512], f32r, tag="encT")
                nc.vector.tensor_copy(encTj[:, :nr], trp[:, :nr])
                for a in range(NA):
                    nc.tensor.matmul(
                        accs[a][:, :nr],
                        wenc_r[:, j, bass.ts(a, 128)],
                        encTj[:, :nr],
                        start=(j == 0),
                        stop=(j == NE - 1),
                    )

            # step 3: relu(enc_attT + dec_attT[:, b]) per batch segment
            combs = [
                combp.tile([128, 512], f32r, tag=f"comb{a}", name=f"comb{a}")
                for a in range(NA)
            ]
            for a in range(NA):
                for b, o0, o1 in _segments(r0, nr):
                    nc.scalar.activation(
                        combs[a][:, o0:o1],
                        accs[a][:, o0:o1],
                        RELU,
                        bias=decatt_sb[:, a, b : b + 1],
                    )

            # step 4: scoresT [1, nr]
            scp = ps_sc.tile([1, 512], f32, tag="sc")
            for a in range(NA):
                nc.tensor.matmul(
                    scp[:, :nr],
                    watt_r[:, a : a + 1],
                    combs[a][:, :nr],
                    start=(a == 0),
                    stop=(a == NA - 1),
                )
            scr_sb = smallp.tile([1, 512], f32, tag="scr")
            nc.vector.tensor_copy(scr_sb[:, :nr], scp[:, :nr])
            if not os.environ.get("SKIP_BOUNCE"):
                nc.sync.dma_start(
                    out=scores_dram[0:1, r0 : r0 + nr], in_=scr_sb[0:1, :nr]
                )

            # repartition scores to row-major [128, ntile] and exponentiate
            if os.environ.get("SKIP_EXP"):
                continue
            scnat = ps_tr.tile([128, 4], f32, tag="tr")
            for t in range(ntile):
                nc.tensor.transpose(
                    scnat[:, t : t + 1], scr_sb[:, bass.ts(t, 128)], identity[:1, :1]
                )
            expnat = smallp.tile([128, 4], f32, tag="expn")
            nc.scalar.activation(expnat[:, :ntile], scnat[:, :ntile], EXP)

            # step 6 (unnormalized): numerator += A_k.T @ enc_tile
            if os.environ.get("SKIP_MASK"):
                continue
            numps = [
                ps_acc.tile([BC, 512], f32, tag=f"acc{jj}", name=f"nump{jj}")
                for jj in range(4)
            ]
            for t in range(ntile):
                k = r0 // 128 + t
                Ak = apool.tile([128, BC], f32r, tag="A")
                nc.vector.tensor_scalar(
                    Ak[:], mb_sb[:, k, :], expnat[:, t : t + 1], None, op0=MULT
                )
                for jj in range(4):
                    nc.tensor.matmul(
                        numps[jj][:],
                        Ak[:],
                        etiles[t][:, bass.ts(jj, 512)],
                        start=(t == 0),
                        stop=(t == ntile - 1),
                    )
            for jj in range(4):
                nc.vector.tensor_add(
                    num_sb[:, bass.ts(jj, 512)],
                    num_sb[:, bass.ts(jj, 512)],
                    numps[jj][:],
                )

        # ---- epilogue: softmax normalizers + outputs ------------------------
        scb = smallp.tile([BC, P], f32, tag="scb")
        nc.sync.dma_start(
            out=scb[:], in_=scores_dram[0].rearrange("(b p) -> b p", b=BC)
        )
        alpha_e = smallp.tile([BC, P], f32, tag="alphe")
        ssum = smallp.tile([BC, 1], f32, tag="ssum")
        nc.scalar.activation(alpha_e[:], scb[:], EXP, accum_out=ssum[:])
        sinv = smallp.tile([BC, 1], f32, tag="sinv")
        nc.vector.reciprocal(sinv[:], ssum[:])
        alpha_sb = smallp.tile([BC, P], f32, tag="alphf")
        nc.vector.tensor_scalar(alpha_sb[:], alpha_e[:], sinv[:], None, op0=MULT)
        nc.sync.dma_start(out=alpha_d.ap(), in_=alpha_sb[:])

        ctx_sb = consts.tile([BC, E], f32)
        nc.vector.tensor_scalar(ctx_sb[:], num_sb[:], sinv[:], None, op0=MULT)
        nc.sync.dma_start(out=ctx_d.ap(), in_=ctx_sb[:])

    nc.compile()
    return nc


def kernel(
    encoder_output,
    decoder_hidden,
    W_enc,
    b_enc,
    W_dec,
    b_dec,
    W_att,
    b_att,
):
    from concourse.bass_utils import run_bass_kernel_spmd

    if "nc" not in _CACHE:
        _CACHE["nc"] = _build_nc()
    nc = _CACHE["nc"]

    encoder_output = np.ascontiguousarray(encoder_output, dtype=np.float32)
    decoder_hidden = np.ascontiguousarray(decoder_hidden, dtype=np.float32)
    weights = {
        "W_enc": np.ascontiguousarray(W_enc, dtype=np.float32),
        "b_enc": np.ascontiguousarray(b_enc, dtype=np.float32),
        "W_dec": np.ascontiguousarray(W_dec, dtype=np.float32),
        "b_dec": np.ascontiguousarray(b_dec, dtype=np.float32),
        "W_att": np.ascontiguousarray(W_att, dtype=np.float32),
    }
    in_maps = []
    for c in range(NCORES):
        sl = slice(c * BC, (c + 1) * BC)
        in_maps.append(
            {
                "encoder_output": encoder_output[sl],
                "decoder_hidden": decoder_hidden[sl],
                **weights,
            }
        )

    res = run_bass_kernel_spmd(nc, in_maps, list(range(NCORES)))
    context = np.concatenate(
        [res.results[c]["context_vector"] for c in range(NCORES)], axis=0
    )
    alpha = np.concatenate([res.results[c]["alpha"] for c in range(NCORES)], axis=0)
    return context.astype(np.float32), alpha.astype(np.float32)
